# revision 4
# baseline (speedup 1.0000x reference)
"""TRN2 Bass kernel for nn_BilinearTensorProduct.

  out = tanh(concat(V1,V2) @ W + einsum('bd,kde,be->bk', V1, T, V2) + sum(b))
  B=8192, D=256, K=64.  Data-parallel: batch sharded 8 ways, T/W/b replicated.

Bilinear path runs as a 3-pass fp16 split matmul (hi/lo, pre-scaled by 2^11 so
the lo parts stay in fp16 normal range; the 2^-22 unscale folds into
affine_mul_reduce's scale). Per (b_tile, k): 6 fp16 matmuls [128d x 128b] @
[128d x 256e] accumulate V1@T[k] in PSUM, then one fused DVE
affine_mul_reduce multiplies by V2 and row-reduces straight into column k of
the per-tile result. The feedforward path is a plain fp32 matmul into one
PSUM bank; the sum(b) constant rides in as an extra contraction block of the
ff matmul (ones-row in CT, sum_b-row in W). tanh runs on the scalar engine.
"""

import numpy as np
from contextlib import ExitStack

import concourse.bacc as bacc
import concourse.mybir as mybir
from concourse import tile
from concourse import bass_utils

B, D, K = 8192, 256, 64
NCORES = 8
BS = B // NCORES          # 1024 batch rows per core
NBT = BS // 128           # 8 b-tiles of 128 rows
NC_FF = 5                 # ff contraction chunks: 4 real + 1 bias block
SCALE = 2048.0            # 2^11 pre-scale for fp16 hi/lo split
UNSCALE = 2.0 ** -22      # undo SCALE^2 after the matmul

f32 = mybir.dt.float32
f16 = mybir.dt.float16

_NC_CACHE = {}


def _build(n_k=K):
    nc = bacc.Bacc("TRN2", target_bir_lowering=False, debug=False)
    A1 = nc.dram_tensor("A1", [D, BS], f16, kind="ExternalInput")
    A2 = nc.dram_tensor("A2", [D, BS], f16, kind="ExternalInput")
    TH = nc.dram_tensor("TH", [K, D, D], f16, kind="ExternalInput")
    TL = nc.dram_tensor("TL", [K, D, D], f16, kind="ExternalInput")
    V2N = nc.dram_tensor("V2N", [BS, D], f32, kind="ExternalInput")
    CT = nc.dram_tensor("CT", [NC_FF * 128, BS], f32, kind="ExternalInput")
    W = nc.dram_tensor("W", [NC_FF * 128, K], f32, kind="ExternalInput")
    OUT = nc.dram_tensor("OUT", [BS, K], f32, kind="ExternalOutput")

    with tile.TileContext(nc) as tc:
        with ExitStack() as ctx:
            const = ctx.enter_context(tc.tile_pool(name="const", bufs=1))
            tpool = ctx.enter_context(tc.tile_pool(name="tpool", bufs=4))
            psb = ctx.enter_context(tc.tile_pool(name="psb", bufs=7, space="PSUM"))
            psff = ctx.enter_context(tc.tile_pool(name="psff", bufs=1, space="PSUM"))
            scr = ctx.enter_context(tc.tile_pool(name="scr", bufs=3))

            a1 = [const.tile([128, BS], f16, name=f"a1_{c}", tag=f"a1_{c}")
                  for c in range(2)]
            a2 = [const.tile([128, BS], f16, name=f"a2_{c}", tag=f"a2_{c}")
                  for c in range(2)]
            v2 = [const.tile([128, D], f32, name=f"v2_{t}", tag=f"v2_{t}")
                  for t in range(NBT)]
            ct = [const.tile([128, BS], f32, name=f"ct_{c}", tag=f"ct_{c}")
                  for c in range(NC_FF)]
            wt = [const.tile([128, K], f32, name=f"wt_{c}", tag=f"wt_{c}")
                  for c in range(NC_FF)]
            bil = [const.tile([128, K], f32, name=f"bil_{t}", tag=f"bil_{t}")
                   for t in range(NBT)]

            for c in range(2):
                nc.sync.dma_start(a1[c][:], A1.ap()[c * 128:(c + 1) * 128, :])
                nc.sync.dma_start(a2[c][:], A2.ap()[c * 128:(c + 1) * 128, :])
            for t in range(NBT):
                nc.sync.dma_start(v2[t][:], V2N.ap()[t * 128:(t + 1) * 128, :])
            for c in range(NC_FF):
                nc.sync.dma_start(ct[c][:], CT.ap()[c * 128:(c + 1) * 128, :])
                nc.sync.dma_start(wt[c][:], W.ap()[c * 128:(c + 1) * 128, :])

            # feedforward (+sum(b) via bias block): all 8 b-tiles in one bank
            ffp = psff.tile([128, NBT * K], f32, name="ffp", tag="ffp")
            for t in range(NBT):
                for c in range(NC_FF):
                    nc.tensor.matmul(
                        ffp[:, t * K:(t + 1) * K],
                        ct[c][:, t * 128:(t + 1) * 128],
                        wt[c][:],
                        start=(c == 0), stop=(c == NC_FF - 1),
                    )

            # bilinear: 3-pass fp16 split
            if n_k < K:  # reduced builds (sim) leave columns unwritten
                for t in range(NBT):
                    nc.any.memset(bil[t][:], 0.0)
            for k in range(n_k):
                th0 = tpool.tile([128, D], f16, tag="th0")
                th1 = tpool.tile([128, D], f16, tag="th1")
                tl0 = tpool.tile([128, D], f16, tag="tl0")
                tl1 = tpool.tile([128, D], f16, tag="tl1")
                nc.sync.dma_start(th0[:], TH.ap()[k, 0:128, :])
                nc.sync.dma_start(th1[:], TH.ap()[k, 128:256, :])
                nc.sync.dma_start(tl0[:], TL.ap()[k, 0:128, :])
                nc.sync.dma_start(tl1[:], TL.ap()[k, 128:256, :])
                for t in range(NBT):
                    bsl = slice(t * 128, (t + 1) * 128)
                    ps = psb.tile([128, D], f32, tag="ps")
                    nc.tensor.matmul(ps[:], a1[0][:, bsl], th0[:], start=True, stop=False)
                    nc.tensor.matmul(ps[:], a1[1][:, bsl], th1[:], start=False, stop=False)
                    nc.tensor.matmul(ps[:], a1[0][:, bsl], tl0[:], start=False, stop=False)
                    nc.tensor.matmul(ps[:], a1[1][:, bsl], tl1[:], start=False, stop=False)
                    nc.tensor.matmul(ps[:], a2[0][:, bsl], th0[:], start=False, stop=False)
                    nc.tensor.matmul(ps[:], a2[1][:, bsl], th1[:], start=False, stop=True)
                    sc = scr.tile([128, D], f32, tag="sc")
                    nc.vector.affine_mul_reduce(
                        out=sc[:], accum_out=bil[t][:, k:k + 1],
                        in0=ps[:], in1=v2[t][:], scale=UNSCALE, bias=0.0,
                    )

            # epilogue: out = tanh(bil + ff)
            for t in range(NBT):
                pre = scr.tile([128, K], f32, tag="pre")
                nc.vector.tensor_tensor(
                    pre[:], bil[t][:], ffp[:, t * K:(t + 1) * K],
                    mybir.AluOpType.add,
                )
                ot = scr.tile([128, K], f32, tag="ot")
                nc.scalar.activation(
                    ot[:], pre[:], mybir.ActivationFunctionType.Tanh,
                )
                nc.sync.dma_start(OUT.ap()[t * 128:(t + 1) * 128, :], ot[:])

    nc.compile()
    return nc


def _prep_inputs(V1, V2, T, W, b):
    V1 = np.asarray(V1, np.float32)
    V2 = np.asarray(V2, np.float32)
    T = np.asarray(T, np.float32)
    W = np.asarray(W, np.float32)
    b = np.asarray(b, np.float32)

    Ts = T * np.float32(SCALE)
    TH = Ts.astype(np.float16)
    TL = (Ts - TH.astype(np.float32)).astype(np.float16)

    V1s = V1 * np.float32(SCALE)
    A1f = V1s.astype(np.float16)
    A2f = (V1s - A1f.astype(np.float32)).astype(np.float16)

    # ff with sum(b) folded in: CT gets a ones-row block, W a sum_b row
    CTf = np.concatenate([V1, V2], axis=1)  # [B, 512]
    sum_b = np.float32(b.sum(dtype=np.float64))
    Wx = np.zeros((NC_FF * 128, K), dtype=np.float32)
    Wx[:512] = W
    Wx[512, :] = sum_b

    in_maps = []
    for c in range(NCORES):
        sl = slice(c * BS, (c + 1) * BS)
        CTx = np.zeros((NC_FF * 128, BS), dtype=np.float32)
        CTx[:512] = CTf[sl].T
        CTx[512, :] = 1.0
        in_maps.append({
            "A1": np.ascontiguousarray(A1f[sl].T),
            "A2": np.ascontiguousarray(A2f[sl].T),
            "TH": TH,
            "TL": TL,
            "V2N": V2[sl],
            "CT": CTx,
            "W": Wx,
        })
    return in_maps


def kernel(V1, V2, T, W, b):
    if "nc" not in _NC_CACHE:
        _NC_CACHE["nc"] = _build()
    nc = _NC_CACHE["nc"]
    in_maps = _prep_inputs(V1, V2, T, W, b)
    res = bass_utils.run_bass_kernel_spmd(nc, in_maps, core_ids=list(range(NCORES)))
    return np.concatenate([r["OUT"] for r in res.results], axis=0)


# revision 5
# speedup vs baseline: 1.0198x; 1.0198x over previous
"""TRN2 Bass kernel for nn_BilinearTensorProduct.

  out = tanh(concat(V1,V2) @ W + einsum('bd,kde,be->bk', V1, T, V2) + sum(b))
  B=8192, D=256, K=64.  Data-parallel: batch sharded 8 ways, T/W/b replicated.

Bilinear path runs as a 3-pass fp16 split matmul (hi/lo, pre-scaled by 2^11 so
the lo parts stay in fp16 normal range; the 2^-22 unscale folds into
affine_mul_reduce's scale). Per (b_tile, k): 6 fp16 matmuls [128d x 128b] @
[128d x 256e] accumulate V1@T[k] in PSUM, then one fused DVE
affine_mul_reduce multiplies by V2 and row-reduces straight into column k of
the per-tile result. The feedforward path is a plain fp32 matmul into one
PSUM bank; the sum(b) constant rides in as an extra contraction block of the
ff matmul (ones-row in CT, sum_b-row in W). tanh runs on the scalar engine.
"""

import numpy as np
from contextlib import ExitStack

import concourse.bacc as bacc
import concourse.mybir as mybir
from concourse import tile
from concourse import bass_utils

B, D, K = 8192, 256, 64
NCORES = 8
BS = B // NCORES          # 1024 batch rows per core
NBT = BS // 128           # 8 b-tiles of 128 rows
NC_FF = 5                 # ff contraction chunks: 4 real + 1 bias block
SCALE = 2048.0            # 2^11 pre-scale for fp16 hi/lo split
UNSCALE = 2.0 ** -22      # undo SCALE^2 after the matmul

f32 = mybir.dt.float32
f16 = mybir.dt.float16

_NC_CACHE = {}


def _build(n_k=K):
    nc = bacc.Bacc("TRN2", target_bir_lowering=False, debug=False)
    A1 = nc.dram_tensor("A1", [D, BS], f16, kind="ExternalInput")
    A2 = nc.dram_tensor("A2", [D, BS], f16, kind="ExternalInput")
    TH = nc.dram_tensor("TH", [K, D, D], f16, kind="ExternalInput")
    TL = nc.dram_tensor("TL", [K, D, D], f16, kind="ExternalInput")
    V2N = nc.dram_tensor("V2N", [BS, D], f32, kind="ExternalInput")
    CT = nc.dram_tensor("CT", [NC_FF * 128, BS], f32, kind="ExternalInput")
    W = nc.dram_tensor("W", [NC_FF * 128, K], f32, kind="ExternalInput")
    OUT = nc.dram_tensor("OUT", [BS, K], f32, kind="ExternalOutput")

    with tile.TileContext(nc) as tc:
        with ExitStack() as ctx:
            const = ctx.enter_context(tc.tile_pool(name="const", bufs=1))
            tpool = ctx.enter_context(tc.tile_pool(name="tpool", bufs=4))
            psb = ctx.enter_context(tc.tile_pool(name="psb", bufs=7, space="PSUM"))
            psff = ctx.enter_context(tc.tile_pool(name="psff", bufs=1, space="PSUM"))
            scr = ctx.enter_context(tc.tile_pool(name="scr", bufs=3))

            a1 = [const.tile([128, BS], f16, name=f"a1_{c}", tag=f"a1_{c}")
                  for c in range(2)]
            a2 = [const.tile([128, BS], f16, name=f"a2_{c}", tag=f"a2_{c}")
                  for c in range(2)]
            v2 = [const.tile([128, D], f32, name=f"v2_{t}", tag=f"v2_{t}")
                  for t in range(NBT)]
            ct = [const.tile([128, BS], f32, name=f"ct_{c}", tag=f"ct_{c}")
                  for c in range(NC_FF)]
            wt = [const.tile([128, K], f32, name=f"wt_{c}", tag=f"wt_{c}")
                  for c in range(NC_FF)]
            bil = [const.tile([128, K], f32, name=f"bil_{t}", tag=f"bil_{t}")
                   for t in range(NBT)]

            for c in range(2):
                nc.sync.dma_start(a1[c][:], A1.ap()[c * 128:(c + 1) * 128, :])
                nc.sync.dma_start(a2[c][:], A2.ap()[c * 128:(c + 1) * 128, :])
            for t in range(NBT):
                nc.sync.dma_start(v2[t][:], V2N.ap()[t * 128:(t + 1) * 128, :])
            for c in range(NC_FF):
                nc.sync.dma_start(ct[c][:], CT.ap()[c * 128:(c + 1) * 128, :])
                nc.sync.dma_start(wt[c][:], W.ap()[c * 128:(c + 1) * 128, :])

            ffp = psff.tile([128, NBT * K], f32, name="ffp", tag="ffp")

            # bilinear: 3-pass fp16 split
            if n_k < K:  # reduced builds (sim) leave columns unwritten
                for t in range(NBT):
                    nc.any.memset(bil[t][:], 0.0)
            for k in range(n_k):
                if k == 2 or (n_k <= 2 and k == n_k - 1):
                    # feedforward (+sum(b) bias block), placed after the PE
                    # pipeline is rolling so its CT DMAs don't gate startup
                    for t in range(NBT):
                        for c in range(NC_FF):
                            nc.tensor.matmul(
                                ffp[:, t * K:(t + 1) * K],
                                ct[c][:, t * 128:(t + 1) * 128],
                                wt[c][:],
                                start=(c == 0), stop=(c == NC_FF - 1),
                            )
                th0 = tpool.tile([128, D], f16, tag="th0")
                th1 = tpool.tile([128, D], f16, tag="th1")
                tl0 = tpool.tile([128, D], f16, tag="tl0")
                tl1 = tpool.tile([128, D], f16, tag="tl1")
                nc.sync.dma_start(th0[:], TH.ap()[k, 0:128, :])
                nc.sync.dma_start(th1[:], TH.ap()[k, 128:256, :])
                nc.sync.dma_start(tl0[:], TL.ap()[k, 0:128, :])
                nc.sync.dma_start(tl1[:], TL.ap()[k, 128:256, :])
                for t in range(NBT):
                    bsl = slice(t * 128, (t + 1) * 128)
                    ps = psb.tile([128, D], f32, tag="ps")
                    nc.tensor.matmul(ps[:], a1[0][:, bsl], th0[:], start=True, stop=False)
                    nc.tensor.matmul(ps[:], a1[1][:, bsl], th1[:], start=False, stop=False)
                    nc.tensor.matmul(ps[:], a1[0][:, bsl], tl0[:], start=False, stop=False)
                    nc.tensor.matmul(ps[:], a1[1][:, bsl], tl1[:], start=False, stop=False)
                    nc.tensor.matmul(ps[:], a2[0][:, bsl], th0[:], start=False, stop=False)
                    nc.tensor.matmul(ps[:], a2[1][:, bsl], th1[:], start=False, stop=True)
                    sc = scr.tile([128, D], f32, tag="sc")
                    nc.vector.affine_mul_reduce(
                        out=sc[:], accum_out=bil[t][:, k:k + 1],
                        in0=ps[:], in1=v2[t][:], scale=UNSCALE, bias=0.0,
                    )

            # epilogue: out = tanh(bil + ff)
            for t in range(NBT):
                pre = scr.tile([128, K], f32, tag="pre")
                nc.vector.tensor_tensor(
                    pre[:], bil[t][:], ffp[:, t * K:(t + 1) * K],
                    mybir.AluOpType.add,
                )
                ot = scr.tile([128, K], f32, tag="ot")
                nc.scalar.activation(
                    ot[:], pre[:], mybir.ActivationFunctionType.Tanh,
                )
                nc.sync.dma_start(OUT.ap()[t * 128:(t + 1) * 128, :], ot[:])

    nc.compile()
    return nc


def _prep_inputs(V1, V2, T, W, b):
    V1 = np.asarray(V1, np.float32)
    V2 = np.asarray(V2, np.float32)
    T = np.asarray(T, np.float32)
    W = np.asarray(W, np.float32)
    b = np.asarray(b, np.float32)

    Ts = T * np.float32(SCALE)
    TH = Ts.astype(np.float16)
    TL = (Ts - TH.astype(np.float32)).astype(np.float16)

    V1s = V1 * np.float32(SCALE)
    A1f = V1s.astype(np.float16)
    A2f = (V1s - A1f.astype(np.float32)).astype(np.float16)

    # ff with sum(b) folded in: CT gets a ones-row block, W a sum_b row
    CTf = np.concatenate([V1, V2], axis=1)  # [B, 512]
    sum_b = np.float32(b.sum(dtype=np.float64))
    Wx = np.zeros((NC_FF * 128, K), dtype=np.float32)
    Wx[:512] = W
    Wx[512, :] = sum_b

    in_maps = []
    for c in range(NCORES):
        sl = slice(c * BS, (c + 1) * BS)
        CTx = np.zeros((NC_FF * 128, BS), dtype=np.float32)
        CTx[:512] = CTf[sl].T
        CTx[512, :] = 1.0
        in_maps.append({
            "A1": np.ascontiguousarray(A1f[sl].T),
            "A2": np.ascontiguousarray(A2f[sl].T),
            "TH": TH,
            "TL": TL,
            "V2N": V2[sl],
            "CT": CTx,
            "W": Wx,
        })
    return in_maps


def kernel(V1, V2, T, W, b):
    if "nc" not in _NC_CACHE:
        _NC_CACHE["nc"] = _build()
    nc = _NC_CACHE["nc"]
    in_maps = _prep_inputs(V1, V2, T, W, b)
    res = bass_utils.run_bass_kernel_spmd(nc, in_maps, core_ids=list(range(NCORES)))
    return np.concatenate([r["OUT"] for r in res.results], axis=0)


# revision 7
# speedup vs baseline: 1.0209x; 1.0010x over previous
"""TRN2 Bass kernel for nn_BilinearTensorProduct.

  out = tanh(concat(V1,V2) @ W + einsum('bd,kde,be->bk', V1, T, V2) + sum(b))
  B=8192, D=256, K=64.  Data-parallel: batch sharded 8 ways, T/W/b replicated.

Bilinear path runs as a 3-pass fp16 split matmul (hi/lo, pre-scaled by 2^11 so
the lo parts stay in fp16 normal range; the 2^-22 unscale folds into
affine_mul_reduce's scale). Per (b_tile, k): 6 fp16 matmuls [128d x 128b] @
[128d x 256e] accumulate V1@T[k] in PSUM, then one fused DVE
affine_mul_reduce multiplies by V2 and row-reduces straight into column k of
the per-tile result. The feedforward path is a plain fp32 matmul into one
PSUM bank; the sum(b) constant rides in as an extra contraction block of the
ff matmul (ones-row in CT, sum_b-row in W). tanh runs on the scalar engine.
"""

import numpy as np
from contextlib import ExitStack

import concourse.bacc as bacc
import concourse.mybir as mybir
from concourse import tile
from concourse import bass_utils

B, D, K = 8192, 256, 64
NCORES = 8
BS = B // NCORES          # 1024 batch rows per core
NBT = BS // 128           # 8 b-tiles of 128 rows
NC_FF = 5                 # ff contraction chunks: 4 real + 1 bias block
SCALE = 2048.0            # 2^11 pre-scale for fp16 hi/lo split
UNSCALE = 2.0 ** -22      # undo SCALE^2 after the matmul

f32 = mybir.dt.float32
f16 = mybir.dt.float16

_NC_CACHE = {}


def _build(n_k=K):
    nc = bacc.Bacc("TRN2", target_bir_lowering=False, debug=False)
    A1 = nc.dram_tensor("A1", [D, BS], f16, kind="ExternalInput")
    A2 = nc.dram_tensor("A2", [D, BS], f16, kind="ExternalInput")
    TH = nc.dram_tensor("TH", [K, D, D], f16, kind="ExternalInput")
    TL = nc.dram_tensor("TL", [K, D, D], f16, kind="ExternalInput")
    V2N = nc.dram_tensor("V2N", [BS, D], f32, kind="ExternalInput")
    CT = nc.dram_tensor("CT", [NC_FF * 128, BS], f32, kind="ExternalInput")
    W = nc.dram_tensor("W", [NC_FF * 128, K], f32, kind="ExternalInput")
    OUT = nc.dram_tensor("OUT", [BS, K], f32, kind="ExternalOutput")

    with tile.TileContext(nc) as tc:
        with ExitStack() as ctx:
            const = ctx.enter_context(tc.tile_pool(name="const", bufs=1))
            tpool = ctx.enter_context(tc.tile_pool(name="tpool", bufs=4))
            psb = ctx.enter_context(tc.tile_pool(name="psb", bufs=7, space="PSUM"))
            psff = ctx.enter_context(tc.tile_pool(name="psff", bufs=1, space="PSUM"))
            scr = ctx.enter_context(tc.tile_pool(name="scr", bufs=3))

            a1 = [const.tile([128, BS], f16, name=f"a1_{c}", tag=f"a1_{c}")
                  for c in range(2)]
            a2 = [const.tile([128, BS], f16, name=f"a2_{c}", tag=f"a2_{c}")
                  for c in range(2)]
            v2 = [const.tile([128, D], f32, name=f"v2_{t}", tag=f"v2_{t}")
                  for t in range(NBT)]
            ct = [const.tile([128, BS], f32, name=f"ct_{c}", tag=f"ct_{c}")
                  for c in range(NC_FF)]
            wt = [const.tile([128, K], f32, name=f"wt_{c}", tag=f"wt_{c}")
                  for c in range(NC_FF)]
            bil = [const.tile([128, K], f32, name=f"bil_{t}", tag=f"bil_{t}")
                   for t in range(NBT)]

            for c in range(2):
                nc.sync.dma_start(a1[c][:], A1.ap()[c * 128:(c + 1) * 128, :])
                nc.sync.dma_start(a2[c][:], A2.ap()[c * 128:(c + 1) * 128, :])

            ffp = psff.tile([128, NBT * K], f32, name="ffp", tag="ffp")

            # bilinear: 3-pass fp16 split
            if n_k < K:  # reduced builds (sim) leave columns unwritten
                for t in range(NBT):
                    nc.any.memset(bil[t][:], 0.0)
            k_ct = 1 if n_k > 1 else 0
            k_ff = 4 if n_k > 4 else max(n_k - 1, 0)
            for k in range(n_k):
                # stagger the non-critical const DMAs behind the first T tiles
                # so the k=0 matmuls aren't queued behind 5MB of input DMA
                if k == k_ct:
                    for c in range(NC_FF):
                        nc.sync.dma_start(ct[c][:], CT.ap()[c * 128:(c + 1) * 128, :])
                        nc.sync.dma_start(wt[c][:], W.ap()[c * 128:(c + 1) * 128, :])
                if k == k_ff:
                    # feedforward (+sum(b) bias block), after the PE is rolling
                    for t in range(NBT):
                        for c in range(NC_FF):
                            nc.tensor.matmul(
                                ffp[:, t * K:(t + 1) * K],
                                ct[c][:, t * 128:(t + 1) * 128],
                                wt[c][:],
                                start=(c == 0), stop=(c == NC_FF - 1),
                            )
                th0 = tpool.tile([128, D], f16, tag="th0")
                th1 = tpool.tile([128, D], f16, tag="th1")
                tl0 = tpool.tile([128, D], f16, tag="tl0")
                tl1 = tpool.tile([128, D], f16, tag="tl1")
                nc.sync.dma_start(th0[:], TH.ap()[k, 0:128, :])
                nc.sync.dma_start(th1[:], TH.ap()[k, 128:256, :])
                nc.sync.dma_start(tl0[:], TL.ap()[k, 0:128, :])
                nc.sync.dma_start(tl1[:], TL.ap()[k, 128:256, :])
                if k == 0:
                    # v2 after the first T tiles in queue order, but still
                    # before its first reader (k=0's affine_mul_reduce)
                    for t in range(NBT):
                        nc.sync.dma_start(v2[t][:], V2N.ap()[t * 128:(t + 1) * 128, :])
                for t in range(NBT):
                    bsl = slice(t * 128, (t + 1) * 128)
                    ps = psb.tile([128, D], f32, tag="ps")
                    nc.tensor.matmul(ps[:], a1[0][:, bsl], th0[:], start=True, stop=False)
                    nc.tensor.matmul(ps[:], a1[1][:, bsl], th1[:], start=False, stop=False)
                    nc.tensor.matmul(ps[:], a1[0][:, bsl], tl0[:], start=False, stop=False)
                    nc.tensor.matmul(ps[:], a1[1][:, bsl], tl1[:], start=False, stop=False)
                    nc.tensor.matmul(ps[:], a2[0][:, bsl], th0[:], start=False, stop=False)
                    nc.tensor.matmul(ps[:], a2[1][:, bsl], th1[:], start=False, stop=True)
                    sc = scr.tile([128, D], f32, tag="sc")
                    nc.vector.affine_mul_reduce(
                        out=sc[:], accum_out=bil[t][:, k:k + 1],
                        in0=ps[:], in1=v2[t][:], scale=UNSCALE, bias=0.0,
                    )

            # epilogue: out = tanh(bil + ff)
            for t in range(NBT):
                pre = scr.tile([128, K], f32, tag="pre")
                nc.vector.tensor_tensor(
                    pre[:], bil[t][:], ffp[:, t * K:(t + 1) * K],
                    mybir.AluOpType.add,
                )
                ot = scr.tile([128, K], f32, tag="ot")
                nc.scalar.activation(
                    ot[:], pre[:], mybir.ActivationFunctionType.Tanh,
                )
                nc.sync.dma_start(OUT.ap()[t * 128:(t + 1) * 128, :], ot[:])

    nc.compile()
    return nc


def _prep_inputs(V1, V2, T, W, b):
    V1 = np.asarray(V1, np.float32)
    V2 = np.asarray(V2, np.float32)
    T = np.asarray(T, np.float32)
    W = np.asarray(W, np.float32)
    b = np.asarray(b, np.float32)

    Ts = T * np.float32(SCALE)
    TH = Ts.astype(np.float16)
    TL = (Ts - TH.astype(np.float32)).astype(np.float16)

    V1s = V1 * np.float32(SCALE)
    A1f = V1s.astype(np.float16)
    A2f = (V1s - A1f.astype(np.float32)).astype(np.float16)

    # ff with sum(b) folded in: CT gets a ones-row block, W a sum_b row
    CTf = np.concatenate([V1, V2], axis=1)  # [B, 512]
    sum_b = np.float32(b.sum(dtype=np.float64))
    Wx = np.zeros((NC_FF * 128, K), dtype=np.float32)
    Wx[:512] = W
    Wx[512, :] = sum_b

    in_maps = []
    for c in range(NCORES):
        sl = slice(c * BS, (c + 1) * BS)
        CTx = np.zeros((NC_FF * 128, BS), dtype=np.float32)
        CTx[:512] = CTf[sl].T
        CTx[512, :] = 1.0
        in_maps.append({
            "A1": np.ascontiguousarray(A1f[sl].T),
            "A2": np.ascontiguousarray(A2f[sl].T),
            "TH": TH,
            "TL": TL,
            "V2N": V2[sl],
            "CT": CTx,
            "W": Wx,
        })
    return in_maps


def kernel(V1, V2, T, W, b):
    if "nc" not in _NC_CACHE:
        _NC_CACHE["nc"] = _build()
    nc = _NC_CACHE["nc"]
    in_maps = _prep_inputs(V1, V2, T, W, b)
    res = bass_utils.run_bass_kernel_spmd(nc, in_maps, core_ids=list(range(NCORES)))
    return np.concatenate([r["OUT"] for r in res.results], axis=0)
